# revision 10
# baseline (speedup 1.0000x reference)
"""nn_DWTFrontEnd Trainium2 Bass kernel — tensor-engine formulation.

The whole 3-level db4 DWT band split is a linear map x -> (band0..band3).
Each band operator M_b is block-banded and 8-periodic: an aligned 128-sample
output tile of a band depends on at most 256 consecutive input samples
(interior row support is +/-49). With the input laid out time-major
([time, signal] blocks of [128, 128]), each output tile of each band is the
PSUM accumulation of exactly TWO bf16 matmuls:

    y_b[:, 128m:128m+128] = X_blk[m]^T @ A0_b[m] + X_blk[m+1]^T @ A1_b[m]

where X_blk[j] = x_ext[128j - 56 : 128j + 72, sigs] (zero-padded outside
[0, N)) sits in SBUF as lhsT (stationary, [128 time, 128 sig]) and the
A matrices ([128 time, 3 bands x 128 cols]) are precomputed on the host from
impulse responses of the exact reference pipeline (symmetric-extension edge
behavior baked into the m=0 / m=63 matrices; all other tiles share one pair).

Bands 0,1,2 are computed on hardware (bf16 in, fp32 PSUM accumulate, bf16
out); band3 = x - band0 - band1 - band2 on the host (exact linearity).
Host transposes x into time-major bf16 and un-shards the result; each of the
8 NeuronCores handles 256 signals (two 128-signal halves) in ONE launch.

Engine budget per core: PE 128 tiles x 2 matmuls x 384 cols; PSUM->SBUF
copies round-robin over DVE/ACT/GPSIMD; DMA ~16.3 MB total.
"""
import sys
for p in ("/opt/trn_rl_repo", "/root/.axon_site/_ro/trn_rl_repo"):
    if p not in sys.path:
        sys.path.append(p)

import numpy as np
import ml_dtypes
import concourse.bass as bass
import concourse.mybir as mybir
import concourse.tile as tile
from concourse.tile_rust import add_dep_helper

F32 = mybir.dt.float32
BF16 = mybir.dt.bfloat16
NPBF16 = ml_dtypes.bfloat16

# ---------------------------------------------------------------------------
# db4 filters (pywt conventions, as in the reference)
REC_LO = np.array([0.23037781330885523, 0.7148465705525415, 0.6308807679295904,
                   -0.027983769416983849, -0.18703481171888114, 0.030841381835986965,
                   0.032883011666982945, -0.010597401784997278], dtype=np.float64)
F = 8
REC_HI = np.array([(-1.0) ** k * REC_LO[F - 1 - k] for k in range(F)], dtype=np.float64)
DEC_LO = REC_LO[::-1].copy()
DEC_HI = REC_HI[::-1].copy()
LEVELS = 3
N = 8192
S = 56                 # left shift of the input block grid (multiple of 8)
NB = N // 128          # 64 output tiles per signal
NBLK = NB + 1          # 65 input blocks (extended length 8320)
HW_BANDS = (0, 1, 2)   # bands computed on hardware; band3 = x - sum on host
NHB = len(HW_BANDS)
W = NHB * 128          # matmul moving width (384)


# ---------------------------------------------------------------------------
# host-side exact reference pipeline (numpy port of the jax reference)
def _conv(x, kern, stride=1, lhs_dilation=1, padding=(0, 0)):
    # out[i] = sum_k xp[i*stride + k] * kern[k]  (correlation, XLA semantics)
    nsig, L = x.shape
    kern = np.asarray(kern, x.dtype)
    if lhs_dilation > 1:
        xd = np.zeros((nsig, (L - 1) * lhs_dilation + 1), dtype=x.dtype)
        xd[:, ::lhs_dilation] = x
    else:
        xd = x
    xp = np.pad(xd, ((0, 0), padding))
    K = len(kern)
    outL = (xp.shape[1] - K) // stride + 1
    out = np.zeros((nsig, outL), dtype=x.dtype)
    for k in range(K):
        out += kern[k] * xp[:, k: k + stride * outL: stride]
    return out


def _dwt(x):
    xe = np.pad(x, ((0, 0), (F - 1, F - 1)), mode='symmetric')[:, 1:]
    return _conv(xe, REC_LO, stride=2), _conv(xe, REC_HI, stride=2)


def _idwt(a, d):
    return (_conv(a, DEC_LO, lhs_dilation=2, padding=(1, 1)) +
            _conv(d, DEC_HI, lhs_dilation=2, padding=(1, 1)))


def _waverec(coeffs):
    a = coeffs[0]
    for d in coeffs[1:]:
        if a.shape[-1] == d.shape[-1] + 1:
            a = a[:, :-1]
        a = _idwt(a, d)
    return a


def _bands(x):
    a = x
    details = []
    for _ in range(LEVELS):
        a, d = _dwt(a)
        details.append(d)
    coeffs = [a] + details[::-1]
    out = []
    for i in range(LEVELS + 1):
        kept = [c if j == i else np.zeros_like(c) for j, c in enumerate(coeffs)]
        out.append(_waverec(kept)[:, :x.shape[1]])
    return out


# ---------------------------------------------------------------------------
# operator construction: columns of M_b via impulse responses
def _build_coef():
    """Returns (coef_host [128, 6*W] bf16, tile->(i0, i1) matrix index map).

    Matrix i layout: [128 time-in-block, NHB*128] fp64 -> bf16.
    Order: 0/1 = tile-0 pair, 2/3 = interior pair, 4/5 = tile-63 pair.
    """
    EL = np.arange(0, 200)           # exact columns for tile 0 (reach <= 175)
    ER = np.arange(N - 184, N)       # exact columns for tile 63 (reach >= 8022)
    PH = np.arange(4096, 4104)       # interior phase templates
    T = np.concatenate([EL, ER, PH])
    E = np.zeros((len(T), N))
    E[np.arange(len(T)), T] = 1.0
    resp = _bands(E)                 # resp[b][i] = column M_b[:, T[i]]
    nEL, nER = len(EL), len(ER)

    def col(b, u):
        R = resp[b]
        if u < nEL:
            return R[u]
        if u >= N - nER:
            return R[nEL + (u - (N - nER))]
        p = u % 8
        tpl = R[nEL + nER + p]
        d = u - (4096 + p)
        out = np.zeros(N)
        if d >= 0:
            out[d:] = tpl[:N - d]
        else:
            out[:d] = tpl[-d:]
        return out

    def tile_pair(m):
        mats = []
        for j in (m, m + 1):
            A = np.zeros((128, W))
            for k in range(128):
                u = -S + 128 * j + k
                if 0 <= u < N:
                    for bi, b in enumerate(HW_BANDS):
                        A[k, bi * 128:(bi + 1) * 128] = \
                            col(b, u)[128 * m:128 * m + 128]
            mats.append(A)
        return mats

    mats = tile_pair(0) + tile_pair(30) + tile_pair(NB - 1)
    coef = np.zeros((128, 6 * W), dtype=NPBF16)
    for i, A in enumerate(mats):
        coef[:, i * W:(i + 1) * W] = A.astype(NPBF16)

    def idx(m):
        if m == 0:
            return 0, 1
        if m == NB - 1:
            return 4, 5
        return 2, 3
    return coef, idx


# ---------------------------------------------------------------------------
def build_kernel():
    coef_host, idx = _build_coef()
    nc = bass.Bass(trn_type="TRN2")
    xt_d = nc.dram_tensor("xt", [128, NBLK * 256], BF16, kind="ExternalInput").ap()
    cf_d = nc.dram_tensor("coef", [128, 6 * W], BF16, kind="ExternalInput").ap()
    y_d = nc.dram_tensor("y", [2, NHB, 128, N], BF16, kind="ExternalOutput").ap()

    sinks = []
    last = {"dve": None, "act": None, "pool": None}
    with tile.TileContext(nc) as tc:
        with tc.tile_pool(name="ded", bufs=1) as ded, \
             tc.tile_pool(name="psum", bufs=8, space="PSUM") as psum:

            coefb = ded.tile([128, 6 * W], BF16, tag="coefb")
            sinks.append(nc.sync.dma_start(coefb, cf_d))

            # input, chunked so matmuls can start early; chunk boundaries in
            # whole blocks (production order pairs tiles m and m+32, so the
            # first chunk must already cover blocks 0..33)
            xtb = ded.tile([128, NBLK * 256], BF16, tag="xtb")
            xcuts = [0, 34, 50, NBLK]
            for c in range(len(xcuts) - 1):
                lo, hi = xcuts[c] * 256, xcuts[c + 1] * 256
                sinks.append(nc.sync.dma_start(xtb[:, lo:hi], xt_d[:, lo:hi]))

            stage = ded.tile([128, 2 * NHB, N], BF16, tag="stage")

            def copy_dve(dst, src):
                last["dve"] = nc.vector.tensor_copy(dst, src)

            def copy_act(dst, src):
                last["act"] = nc.scalar.copy(dst, src)

            # NOTE: GPSIMD cannot access PSUM, so only DVE + ACT copy out
            copy_fns = [copy_dve, copy_act]

            # A DMA descriptor supports only ONE sync-wait command, so each
            # DMA'd stage region must have a single-engine writer set.
            # Produce tiles in order (0,32),(1,33),... so the copies
            # alternate DVE/ACT per PSUM drain (both engines stay busy)
            # while DVE exclusively writes tiles 0..31 (stage cols
            # [0,4096)) and ACT writes tiles 32..63 ([4096,8192)).
            for h in (0, 1):
                for mm in range(NB // 2):
                    for base, cp in ((0, copy_fns[0]), (NB // 2, copy_fns[1])):
                        m = base + mm
                        i0, i1 = idx(m)
                        P = psum.tile([128, 4, 128], F32, tag="ps")
                        lhs0 = xtb[:, m * 256 + h * 128:
                                   m * 256 + h * 128 + 128]
                        lhs1 = xtb[:, (m + 1) * 256 + h * 128:
                                   (m + 1) * 256 + h * 128 + 128]
                        nc.tensor.matmul(P[:, 0:NHB, :], lhs0,
                                         coefb[:, i0 * W:(i0 + 1) * W],
                                         start=True, stop=False)
                        nc.tensor.matmul(P[:, 0:NHB, :], lhs1,
                                         coefb[:, i1 * W:(i1 + 1) * W],
                                         start=False, stop=True)
                        dst = stage[:, NHB * h:NHB * h + NHB,
                                    128 * m:128 * m + 128]
                        cp(dst, P[:, 0:NHB, :])
                # stream this half's bands out; chunk [0,4096) was written
                # only by DVE, [4096,8192) only by ACT.  Issue from GPSIMD
                # (SWDGE): its waits ride on the Pool engine instruction,
                # which has a real wait budget, unlike the HWDGE descriptor's
                # single sync-wait slot; the Pool engine is otherwise idle.
                for b in range(NHB):
                    for c in (0, 1):
                        sl = slice(c * (N // 2), (c + 1) * (N // 2))
                        sinks.append(nc.gpsimd.dma_start(
                            y_d[h][b][:, sl], stage[:, NHB * h + b, sl]))

            # tail: absorb every outstanding proc onto SP, one nop each
            tc.no_sync_barrier()
            for s in sinks + [v for v in last.values() if v is not None]:
                nop = nc.sync.nop()
                add_dep_helper(nop.ins, s.ins, reason="tail absorb")
    return nc


_CACHE = {}


def _get_kernel():
    if "nc" not in _CACHE:
        _CACHE["nc"] = build_kernel()
    return _CACHE["nc"]


def run_full(x_full, trace=False):
    """x_full: (32, 64, 8192) f32 -> tuple of 4 bands, each (32,64,8192)."""
    from concourse.bass_utils import run_bass_kernel_spmd
    B, C, n = x_full.shape
    assert n == N
    nsig = B * C
    n_cores = 8
    per_core = nsig // n_cores          # 256
    xf = np.ascontiguousarray(x_full.reshape(nsig, n)).astype(np.float32)

    # extended, time-major, bf16: xe[sig, u+S] ; blocks of 128 time samples
    xe = np.zeros((nsig, NBLK * 128), dtype=NPBF16)
    xe[:, S:S + N] = xf.astype(NPBF16)
    # xt_all[p, j, sig] = xe[sig, 128j + p]
    xt_all = np.ascontiguousarray(
        xe.reshape(nsig, NBLK, 128).transpose(2, 1, 0))

    nc = _get_kernel()
    coef_host = _CACHE.setdefault("coef", _build_coef()[0])
    in_maps = []
    for i in range(n_cores):
        sl = slice(i * per_core, (i + 1) * per_core)
        in_maps.append({
            "xt": np.ascontiguousarray(xt_all[:, :, sl]).reshape(128, NBLK * 256),
            "coef": coef_host,
        })
    res = run_bass_kernel_spmd(nc, in_maps, core_ids=list(range(n_cores)),
                               trace=trace)

    bands = np.empty((4, nsig, n), dtype=np.float32)
    for i in range(n_cores):
        y = np.asarray(res.results[i]["y"]).astype(np.float32)  # [2,NHB,128,N]
        base = i * per_core
        for h in (0, 1):
            r = slice(base + h * 128, base + h * 128 + 128)
            for bi, b in enumerate(HW_BANDS):
                bands[b, r, :] = y[h, bi]
    bands[3] = xf - bands[0] - bands[1] - bands[2]
    out = tuple(bands[j].reshape(B, C, n) for j in range(4))
    return out, res


def kernel(x):
    out, _ = run_full(np.asarray(x))
    return out


# revision 22
# speedup vs baseline: 1.2094x; 1.2094x over previous
"""nn_DWTFrontEnd Trainium2 Bass kernel — tensor-engine formulation.

The whole 3-level db4 DWT band split is a linear map x -> (band0..band3).
Each band operator M_b is block-banded and 8-periodic: an aligned 128-sample
output tile of a band depends on at most 256 consecutive input samples
(interior row support is +/-49). With the input laid out time-major
([time, signal] blocks of [128, 128]), each output tile of each band is the
PSUM accumulation of exactly TWO bf16 matmuls:

    y_b[:, 128m:128m+128] = X_blk[m]^T @ A0_b[m] + X_blk[m+1]^T @ A1_b[m]

where X_blk[j] = x_ext[128j - 56 : 128j + 72, sigs] (zero-padded outside
[0, N)) sits in SBUF as lhsT (stationary, [128 time, 128 sig]) and the
A matrices ([128 time, 3 bands x 128 cols]) are precomputed on the host from
impulse responses of the exact reference pipeline (symmetric-extension edge
behavior baked into the m=0 / m=63 matrices; all other tiles share one pair).

Bands 0,1,2 are computed on hardware (bf16 in, fp32 PSUM accumulate, bf16
out); band3 = x - band0 - band1 - band2 on the host (exact linearity).
Host transposes x into time-major bf16 and un-shards the result; each of the
8 NeuronCores handles 256 signals (two 128-signal halves) in ONE launch.

Engine budget per core: PE 128 tiles x 2 matmuls x 384 cols; PSUM->SBUF
copies round-robin over DVE/ACT/GPSIMD; DMA ~16.3 MB total.
"""
import sys
for p in ("/opt/trn_rl_repo", "/root/.axon_site/_ro/trn_rl_repo"):
    if p not in sys.path:
        sys.path.append(p)

import numpy as np
import ml_dtypes
import concourse.bass as bass
import concourse.mybir as mybir
import concourse.tile as tile
from concourse.tile_rust import add_dep_helper

F32 = mybir.dt.float32
BF16 = mybir.dt.bfloat16
NPBF16 = ml_dtypes.bfloat16

# ---------------------------------------------------------------------------
# db4 filters (pywt conventions, as in the reference)
REC_LO = np.array([0.23037781330885523, 0.7148465705525415, 0.6308807679295904,
                   -0.027983769416983849, -0.18703481171888114, 0.030841381835986965,
                   0.032883011666982945, -0.010597401784997278], dtype=np.float64)
F = 8
REC_HI = np.array([(-1.0) ** k * REC_LO[F - 1 - k] for k in range(F)], dtype=np.float64)
DEC_LO = REC_LO[::-1].copy()
DEC_HI = REC_HI[::-1].copy()
LEVELS = 3
N = 8192
S = 56                 # left shift of the input block grid (multiple of 8)
NB = N // 128          # 64 output tiles per signal
NBLK = NB + 1          # 65 input blocks (extended length 8320)
HW_BANDS = (0, 1, 2)   # bands computed on hardware; band3 = x - sum on host
NHB = len(HW_BANDS)
W = NHB * 128          # matmul moving width (384)


# ---------------------------------------------------------------------------
# host-side exact reference pipeline (numpy port of the jax reference)
def _conv(x, kern, stride=1, lhs_dilation=1, padding=(0, 0)):
    # out[i] = sum_k xp[i*stride + k] * kern[k]  (correlation, XLA semantics)
    nsig, L = x.shape
    kern = np.asarray(kern, x.dtype)
    if lhs_dilation > 1:
        xd = np.zeros((nsig, (L - 1) * lhs_dilation + 1), dtype=x.dtype)
        xd[:, ::lhs_dilation] = x
    else:
        xd = x
    xp = np.pad(xd, ((0, 0), padding))
    K = len(kern)
    outL = (xp.shape[1] - K) // stride + 1
    out = np.zeros((nsig, outL), dtype=x.dtype)
    for k in range(K):
        out += kern[k] * xp[:, k: k + stride * outL: stride]
    return out


def _dwt(x):
    xe = np.pad(x, ((0, 0), (F - 1, F - 1)), mode='symmetric')[:, 1:]
    return _conv(xe, REC_LO, stride=2), _conv(xe, REC_HI, stride=2)


def _idwt(a, d):
    return (_conv(a, DEC_LO, lhs_dilation=2, padding=(1, 1)) +
            _conv(d, DEC_HI, lhs_dilation=2, padding=(1, 1)))


def _waverec(coeffs):
    a = coeffs[0]
    for d in coeffs[1:]:
        if a.shape[-1] == d.shape[-1] + 1:
            a = a[:, :-1]
        a = _idwt(a, d)
    return a


def _bands(x):
    a = x
    details = []
    for _ in range(LEVELS):
        a, d = _dwt(a)
        details.append(d)
    coeffs = [a] + details[::-1]
    out = []
    for i in range(LEVELS + 1):
        kept = [c if j == i else np.zeros_like(c) for j, c in enumerate(coeffs)]
        out.append(_waverec(kept)[:, :x.shape[1]])
    return out


# ---------------------------------------------------------------------------
# operator construction: columns of M_b via impulse responses
def _build_coef():
    """Returns (coef_host [128, 6*W] bf16, tile->(i0, i1) matrix index map).

    Matrix i layout: [128 time-in-block, NHB*128] fp64 -> bf16.
    Order: 0/1 = tile-0 pair, 2/3 = interior pair, 4/5 = tile-63 pair.
    """
    EL = np.arange(0, 200)           # exact columns for tile 0 (reach <= 175)
    ER = np.arange(N - 184, N)       # exact columns for tile 63 (reach >= 8022)
    PH = np.arange(4096, 4104)       # interior phase templates
    T = np.concatenate([EL, ER, PH])
    E = np.zeros((len(T), N))
    E[np.arange(len(T)), T] = 1.0
    resp = _bands(E)                 # resp[b][i] = column M_b[:, T[i]]
    nEL, nER = len(EL), len(ER)

    def col(b, u):
        R = resp[b]
        if u < nEL:
            return R[u]
        if u >= N - nER:
            return R[nEL + (u - (N - nER))]
        p = u % 8
        tpl = R[nEL + nER + p]
        d = u - (4096 + p)
        out = np.zeros(N)
        if d >= 0:
            out[d:] = tpl[:N - d]
        else:
            out[:d] = tpl[-d:]
        return out

    def tile_pair(m):
        mats = []
        for j in (m, m + 1):
            A = np.zeros((128, W))
            for k in range(128):
                u = -S + 128 * j + k
                if 0 <= u < N:
                    for bi, b in enumerate(HW_BANDS):
                        A[k, bi * 128:(bi + 1) * 128] = \
                            col(b, u)[128 * m:128 * m + 128]
            mats.append(A)
        return mats

    mats = tile_pair(0) + tile_pair(30) + tile_pair(NB - 1)
    coef = np.zeros((128, 6 * W), dtype=NPBF16)
    for i, A in enumerate(mats):
        coef[:, i * W:(i + 1) * W] = A.astype(NPBF16)

    def idx(m):
        if m == 0:
            return 0, 1
        if m == NB - 1:
            return 4, 5
        return 2, 3
    return coef, idx


# ---------------------------------------------------------------------------
def build_kernel():
    coef_host, idx = _build_coef()
    nc = bass.Bass(trn_type="TRN2")
    xt_d = nc.dram_tensor("xt", [128, NBLK * 256], BF16, kind="ExternalInput").ap()
    cf_d = nc.dram_tensor("coef", [128, 6 * W], BF16, kind="ExternalInput").ap()
    # partition-major band layout so a [128][3][cols] DMA needs no transpose
    y_d = nc.dram_tensor("y", [2, 128, NHB, N], BF16, kind="ExternalOutput").ap()

    sinks = []
    last = {"dve": None, "act": None, "pool": None, "pe": None}
    with tile.TileContext(nc) as tc:
        with tc.tile_pool(name="ded", bufs=1) as ded, \
             tc.tile_pool(name="psum", bufs=8, space="PSUM") as psum:

            coefb = ded.tile([128, 6 * W], BF16, tag="coefb")
            sinks.append(nc.sync.dma_start(coefb, cf_d))

            # input, chunked so matmuls can start early; chunk boundaries in
            # whole blocks (tile group g needs blocks up to 16g+17)
            xtb = ded.tile([128, NBLK * 256], BF16, tag="xtb")
            xcuts = [0, 18, 34, 50, NBLK]
            for c in range(len(xcuts) - 1):
                lo, hi = xcuts[c] * 256, xcuts[c + 1] * 256
                sinks.append(nc.sync.dma_start(xtb[:, lo:hi], xt_d[:, lo:hi]))

            stage = ded.tile([128, 2 * NHB, N], BF16, tag="stage")
            scratch = ded.tile([128, 64], F32, tag="scratch")

            def copy_dve(dst, src):
                last["dve"] = nc.vector.tensor_copy(dst, src)
                return last["dve"]

            def copy_act(dst, src):
                last["act"] = nc.scalar.copy(dst, src)
                return last["act"]

            # NOTE: GPSIMD cannot access PSUM, so only DVE + ACT copy out
            copy_fns = [copy_dve, copy_act]

            # A DMA descriptor supports only ONE sync-wait command, so each
            # DMA'd stage region must have a single-engine writer set.
            # Tiles are produced in groups of 16 as pairs (16g+i, 16g+8+i):
            # copies alternate DVE/ACT per PSUM drain (both engines busy)
            # while DVE exclusively writes tiles [16g,16g+8) and ACT
            # [16g+8,16g+16); each group's two engine-regions DMA out as
            # soon as the group completes.  Out-DMAs issue from GPSIMD
            # (SWDGE): its waits ride on the Pool engine instruction, which
            # has a real wait budget, unlike the HWDGE descriptor's single
            # sync-wait slot; the Pool engine is otherwise idle.
            for h in (0, 1):
                for g in range(NB // 16):
                    group_last = {}
                    for i in range(8):
                        for base, cp in ((0, copy_fns[0]), (8, copy_fns[1])):
                            m = 16 * g + base + i
                            i0, i1 = idx(m)
                            P = psum.tile([128, 4, 128], F32, tag="ps")
                            lhs0 = xtb[:, m * 256 + h * 128:
                                       m * 256 + h * 128 + 128]
                            lhs1 = xtb[:, (m + 1) * 256 + h * 128:
                                       (m + 1) * 256 + h * 128 + 128]
                            nc.tensor.matmul(P[:, 0:NHB, :], lhs0,
                                             coefb[:, i0 * W:(i0 + 1) * W],
                                             start=True, stop=False)
                            last["pe"] = nc.tensor.matmul(
                                P[:, 0:NHB, :], lhs1,
                                coefb[:, i1 * W:(i1 + 1) * W],
                                start=False, stop=True)
                            dst = stage[:, NHB * h:NHB * h + NHB,
                                        128 * m:128 * m + 128]
                            group_last[base] = cp(dst, P[:, 0:NHB, :])
                    # Pool fences: absorb each engine's sem wait onto a Pool
                    # ENGINE instruction so the group's DMAs only ever carry
                    # the SWDGE ring-reuse wait — both the DMA descriptor
                    # and GPSIMD instructions have a single sync-wait slot.
                    for fi, cp_ins in enumerate(group_last.values()):
                        col = (2 * h + fi) * (NB // 16) + g  # unique per fence
                        fence = nc.gpsimd.memset(scratch[:, col:col + 1], 0.0)
                        add_dep_helper(fence.ins, cp_ins.ins, reason="grp fence")
                        last["pool"] = fence
                    for half_off in (0, 1024):  # DVE region, ACT region
                        sl = slice(2048 * g + half_off,
                                   2048 * g + half_off + 1024)
                        sinks.append(nc.gpsimd.dma_start(
                            y_d[h][:, :, sl], stage[:, NHB * h:NHB * h + NHB, sl]))

            # tail: absorb every outstanding proc onto SP, one nop each
            tc.no_sync_barrier()
            for s in sinks + [v for v in last.values() if v is not None]:
                nop = nc.sync.nop()
                add_dep_helper(nop.ins, s.ins, reason="tail absorb")
    return nc


_CACHE = {}


def _get_kernel():
    if "nc" not in _CACHE:
        _CACHE["nc"] = build_kernel()
    return _CACHE["nc"]


def run_full(x_full, trace=False):
    """x_full: (32, 64, 8192) f32 -> tuple of 4 bands, each (32,64,8192)."""
    from concourse.bass_utils import run_bass_kernel_spmd
    B, C, n = x_full.shape
    assert n == N
    nsig = B * C
    n_cores = 8
    per_core = nsig // n_cores          # 256
    xf = np.ascontiguousarray(x_full.reshape(nsig, n)).astype(np.float32)

    # extended, time-major, bf16: xe[sig, u+S] ; blocks of 128 time samples
    xe = np.zeros((nsig, NBLK * 128), dtype=NPBF16)
    xe[:, S:S + N] = xf.astype(NPBF16)
    # xt_all[p, j, sig] = xe[sig, 128j + p]
    xt_all = np.ascontiguousarray(
        xe.reshape(nsig, NBLK, 128).transpose(2, 1, 0))

    nc = _get_kernel()
    coef_host = _CACHE.setdefault("coef", _build_coef()[0])
    in_maps = []
    for i in range(n_cores):
        sl = slice(i * per_core, (i + 1) * per_core)
        in_maps.append({
            "xt": np.ascontiguousarray(xt_all[:, :, sl]).reshape(128, NBLK * 256),
            "coef": coef_host,
        })
    res = run_bass_kernel_spmd(nc, in_maps, core_ids=list(range(n_cores)),
                               trace=trace)

    bands = np.empty((4, nsig, n), dtype=np.float32)
    for i in range(n_cores):
        y = np.asarray(res.results[i]["y"]).astype(np.float32)  # [2,128,NHB,N]
        base = i * per_core
        for h in (0, 1):
            r = slice(base + h * 128, base + h * 128 + 128)
            for bi, b in enumerate(HW_BANDS):
                bands[b, r, :] = y[h, :, bi]
    bands[3] = xf - bands[0] - bands[1] - bands[2]
    out = tuple(bands[j].reshape(B, C, n) for j in range(4))
    return out, res


def kernel(x):
    out, _ = run_full(np.asarray(x))
    return out


# revision 25
# speedup vs baseline: 1.2811x; 1.0593x over previous
"""nn_DWTFrontEnd Trainium2 Bass kernel — tensor-engine formulation.

The whole 3-level db4 DWT band split is a linear map x -> (band0..band3).
Each band operator M_b is block-banded and 8-periodic: an aligned 128-sample
output tile of a band depends on at most 256 consecutive input samples
(interior row support is +/-49). With the input laid out time-major
([time, signal] blocks of [128, 128]), each output tile of each band is the
PSUM accumulation of exactly TWO bf16 matmuls:

    y_b[:, 128m:128m+128] = X_blk[m]^T @ A0_b[m] + X_blk[m+1]^T @ A1_b[m]

where X_blk[j] = x_ext[128j - 56 : 128j + 72, sigs] (zero-padded outside
[0, N)) sits in SBUF as lhsT (stationary, [128 time, 128 sig]) and the
A matrices ([128 time, 3 bands x 128 cols]) are precomputed on the host from
impulse responses of the exact reference pipeline (symmetric-extension edge
behavior baked into the m=0 / m=63 matrices; all other tiles share one pair).

Bands 0,1,2 are computed on hardware (bf16 in, fp32 PSUM accumulate, bf16
out); band3 = x - band0 - band1 - band2 on the host (exact linearity).
Host transposes x into time-major bf16 and un-shards the result; each of the
8 NeuronCores handles 256 signals (two 128-signal halves) in ONE launch.

Engine budget per core: PE 128 tiles x 2 matmuls x 384 cols; PSUM->SBUF
copies round-robin over DVE/ACT/GPSIMD; DMA ~16.3 MB total.
"""
import sys
for p in ("/opt/trn_rl_repo", "/root/.axon_site/_ro/trn_rl_repo"):
    if p not in sys.path:
        sys.path.append(p)

import numpy as np
import ml_dtypes
import concourse.bass as bass
import concourse.mybir as mybir
import concourse.tile as tile
from concourse.tile_rust import add_dep_helper

F32 = mybir.dt.float32
BF16 = mybir.dt.bfloat16
NPBF16 = ml_dtypes.bfloat16

# ---------------------------------------------------------------------------
# db4 filters (pywt conventions, as in the reference)
REC_LO = np.array([0.23037781330885523, 0.7148465705525415, 0.6308807679295904,
                   -0.027983769416983849, -0.18703481171888114, 0.030841381835986965,
                   0.032883011666982945, -0.010597401784997278], dtype=np.float64)
F = 8
REC_HI = np.array([(-1.0) ** k * REC_LO[F - 1 - k] for k in range(F)], dtype=np.float64)
DEC_LO = REC_LO[::-1].copy()
DEC_HI = REC_HI[::-1].copy()
LEVELS = 3
N = 8192
S = 56                 # left shift of the input block grid (multiple of 8)
NB = N // 128          # 64 output tiles per signal
NBLK = NB + 1          # 65 input blocks (extended length 8320)
HW_BANDS = (0, 1, 2)   # bands computed on hardware; band3 = x - sum on host
NHB = len(HW_BANDS)
W = NHB * 128          # matmul moving width (384)


# ---------------------------------------------------------------------------
# host-side exact reference pipeline (numpy port of the jax reference)
def _conv(x, kern, stride=1, lhs_dilation=1, padding=(0, 0)):
    # out[i] = sum_k xp[i*stride + k] * kern[k]  (correlation, XLA semantics)
    nsig, L = x.shape
    kern = np.asarray(kern, x.dtype)
    if lhs_dilation > 1:
        xd = np.zeros((nsig, (L - 1) * lhs_dilation + 1), dtype=x.dtype)
        xd[:, ::lhs_dilation] = x
    else:
        xd = x
    xp = np.pad(xd, ((0, 0), padding))
    K = len(kern)
    outL = (xp.shape[1] - K) // stride + 1
    out = np.zeros((nsig, outL), dtype=x.dtype)
    for k in range(K):
        out += kern[k] * xp[:, k: k + stride * outL: stride]
    return out


def _dwt(x):
    xe = np.pad(x, ((0, 0), (F - 1, F - 1)), mode='symmetric')[:, 1:]
    return _conv(xe, REC_LO, stride=2), _conv(xe, REC_HI, stride=2)


def _idwt(a, d):
    return (_conv(a, DEC_LO, lhs_dilation=2, padding=(1, 1)) +
            _conv(d, DEC_HI, lhs_dilation=2, padding=(1, 1)))


def _waverec(coeffs):
    a = coeffs[0]
    for d in coeffs[1:]:
        if a.shape[-1] == d.shape[-1] + 1:
            a = a[:, :-1]
        a = _idwt(a, d)
    return a


def _bands(x):
    a = x
    details = []
    for _ in range(LEVELS):
        a, d = _dwt(a)
        details.append(d)
    coeffs = [a] + details[::-1]
    out = []
    for i in range(LEVELS + 1):
        kept = [c if j == i else np.zeros_like(c) for j, c in enumerate(coeffs)]
        out.append(_waverec(kept)[:, :x.shape[1]])
    return out


# ---------------------------------------------------------------------------
# operator construction: columns of M_b via impulse responses
def _build_coef():
    """Returns (coef_host [128, 6*W] bf16, tile->(i0, i1) matrix index map).

    Matrix i layout: [128 time-in-block, NHB*128] fp64 -> bf16.
    Order: 0/1 = tile-0 pair, 2/3 = interior pair, 4/5 = tile-63 pair.
    """
    EL = np.arange(0, 200)           # exact columns for tile 0 (reach <= 175)
    ER = np.arange(N - 184, N)       # exact columns for tile 63 (reach >= 8022)
    PH = np.arange(4096, 4104)       # interior phase templates
    T = np.concatenate([EL, ER, PH])
    E = np.zeros((len(T), N))
    E[np.arange(len(T)), T] = 1.0
    resp = _bands(E)                 # resp[b][i] = column M_b[:, T[i]]
    nEL, nER = len(EL), len(ER)

    def col(b, u):
        R = resp[b]
        if u < nEL:
            return R[u]
        if u >= N - nER:
            return R[nEL + (u - (N - nER))]
        p = u % 8
        tpl = R[nEL + nER + p]
        d = u - (4096 + p)
        out = np.zeros(N)
        if d >= 0:
            out[d:] = tpl[:N - d]
        else:
            out[:d] = tpl[-d:]
        return out

    def tile_pair(m):
        mats = []
        for j in (m, m + 1):
            A = np.zeros((128, W))
            for k in range(128):
                u = -S + 128 * j + k
                if 0 <= u < N:
                    for bi, b in enumerate(HW_BANDS):
                        A[k, bi * 128:(bi + 1) * 128] = \
                            col(b, u)[128 * m:128 * m + 128]
            mats.append(A)
        return mats

    mats = tile_pair(0) + tile_pair(30) + tile_pair(NB - 1)
    coef = np.zeros((128, 6 * W), dtype=NPBF16)
    for i, A in enumerate(mats):
        coef[:, i * W:(i + 1) * W] = A.astype(NPBF16)

    def idx(m):
        if m == 0:
            return 0, 1
        if m == NB - 1:
            return 4, 5
        return 2, 3
    return coef, idx


# ---------------------------------------------------------------------------
def build_kernel():
    coef_host, idx = _build_coef()
    nc = bass.Bass(trn_type="TRN2")
    xt_d = nc.dram_tensor("xt", [128, NBLK * 256], BF16, kind="ExternalInput").ap()
    cf_d = nc.dram_tensor("coef", [128, 6 * W], BF16, kind="ExternalInput").ap()
    # partition-major band layout so a [128][3][cols] DMA needs no transpose
    y_d = nc.dram_tensor("y", [2, 128, NHB, N], BF16, kind="ExternalOutput").ap()

    sinks = []
    last = {"dve": None, "act": None, "pool": None, "pe": None}
    with tile.TileContext(nc) as tc:
        with tc.tile_pool(name="ded", bufs=1) as ded, \
             tc.tile_pool(name="psum", bufs=8, space="PSUM") as psum:

            coefb = ded.tile([128, 6 * W], BF16, tag="coefb")
            sinks.append(nc.sync.dma_start(coefb, cf_d))

            # input, chunked so matmuls can start early; chunk boundaries in
            # whole blocks (tile group g needs blocks up to 16g+17; the
            # first pair (0,8) only needs blocks 0..10)
            xtb = ded.tile([128, NBLK * 256], BF16, tag="xtb")
            xcuts = [0, 11, 18, 34, 50, NBLK]
            for c in range(len(xcuts) - 1):
                lo, hi = xcuts[c] * 256, xcuts[c + 1] * 256
                sinks.append(nc.sync.dma_start(xtb[:, lo:hi], xt_d[:, lo:hi]))

            stage = ded.tile([128, 2 * NHB, N], BF16, tag="stage")
            scratch = ded.tile([128, 64], F32, tag="scratch")

            def copy_dve(dst, src):
                last["dve"] = nc.vector.tensor_copy(dst, src)
                return last["dve"]

            def copy_act(dst, src):
                last["act"] = nc.scalar.copy(dst, src)
                return last["act"]

            # NOTE: GPSIMD cannot access PSUM, so only DVE + ACT copy out
            copy_fns = [copy_dve, copy_act]

            # A DMA descriptor supports only ONE sync-wait command, so each
            # DMA'd stage region must have a single-engine writer set.
            # Tiles are produced in groups of 16 as pairs (16g+i, 16g+8+i):
            # copies alternate DVE/ACT per PSUM drain (both engines busy)
            # while DVE exclusively writes tiles [16g,16g+8) and ACT
            # [16g+8,16g+16); each group's two engine-regions DMA out as
            # soon as the group completes.  Out-DMAs issue from GPSIMD
            # (SWDGE): its waits ride on the Pool engine instruction, which
            # has a real wait budget, unlike the HWDGE descriptor's single
            # sync-wait slot; the Pool engine is otherwise idle.
            # groups of 16 tiles, except the last two groups of 8 so the
            # final out-DMA tail after the last matmul is half as long
            groups = [(0, 16), (16, 16), (32, 16), (48, 8), (56, 8)]
            for h in (0, 1):
                for gi, (g0, gsz) in enumerate(groups):
                    group_last = {}
                    for i in range(gsz // 2):
                        for base, cp in ((0, copy_fns[0]),
                                         (gsz // 2, copy_fns[1])):
                            m = g0 + base + i
                            i0, i1 = idx(m)
                            P = psum.tile([128, 4, 128], F32, tag="ps")
                            lhs0 = xtb[:, m * 256 + h * 128:
                                       m * 256 + h * 128 + 128]
                            lhs1 = xtb[:, (m + 1) * 256 + h * 128:
                                       (m + 1) * 256 + h * 128 + 128]
                            nc.tensor.matmul(P[:, 0:NHB, :], lhs0,
                                             coefb[:, i0 * W:(i0 + 1) * W],
                                             start=True, stop=False)
                            last["pe"] = nc.tensor.matmul(
                                P[:, 0:NHB, :], lhs1,
                                coefb[:, i1 * W:(i1 + 1) * W],
                                start=False, stop=True)
                            dst = stage[:, NHB * h:NHB * h + NHB,
                                        128 * m:128 * m + 128]
                            group_last[base] = cp(dst, P[:, 0:NHB, :])
                    # Pool fences: absorb each engine's sem wait onto a Pool
                    # ENGINE instruction so the group's DMAs only ever carry
                    # the SWDGE ring-reuse wait — both the DMA descriptor
                    # and GPSIMD instructions have a single sync-wait slot.
                    for fi, cp_ins in enumerate(group_last.values()):
                        col = (2 * h + fi) * len(groups) + gi  # unique/fence
                        fence = nc.gpsimd.memset(scratch[:, col:col + 1], 0.0)
                        add_dep_helper(fence.ins, cp_ins.ins, reason="grp fence")
                        last["pool"] = fence
                    for half_off in (0, 64 * gsz):  # DVE region, ACT region
                        sl = slice(128 * g0 + half_off,
                                   128 * g0 + half_off + 64 * gsz)
                        sinks.append(nc.gpsimd.dma_start(
                            y_d[h][:, :, sl], stage[:, NHB * h:NHB * h + NHB, sl]))

            # tail: absorb every outstanding proc onto SP, one nop each
            tc.no_sync_barrier()
            for s in sinks + [v for v in last.values() if v is not None]:
                nop = nc.sync.nop()
                add_dep_helper(nop.ins, s.ins, reason="tail absorb")
    return nc


_CACHE = {}


def _get_kernel():
    if "nc" not in _CACHE:
        _CACHE["nc"] = build_kernel()
    return _CACHE["nc"]


def run_full(x_full, trace=False):
    """x_full: (32, 64, 8192) f32 -> tuple of 4 bands, each (32,64,8192)."""
    from concourse.bass_utils import run_bass_kernel_spmd
    B, C, n = x_full.shape
    assert n == N
    nsig = B * C
    n_cores = 8
    per_core = nsig // n_cores          # 256
    xf = np.ascontiguousarray(x_full.reshape(nsig, n)).astype(np.float32)

    # extended, time-major, bf16: xe[sig, u+S] ; blocks of 128 time samples
    xe = np.zeros((nsig, NBLK * 128), dtype=NPBF16)
    xe[:, S:S + N] = xf.astype(NPBF16)
    # xt_all[p, j, sig] = xe[sig, 128j + p]
    xt_all = np.ascontiguousarray(
        xe.reshape(nsig, NBLK, 128).transpose(2, 1, 0))

    nc = _get_kernel()
    coef_host = _CACHE.setdefault("coef", _build_coef()[0])
    in_maps = []
    for i in range(n_cores):
        sl = slice(i * per_core, (i + 1) * per_core)
        in_maps.append({
            "xt": np.ascontiguousarray(xt_all[:, :, sl]).reshape(128, NBLK * 256),
            "coef": coef_host,
        })
    res = run_bass_kernel_spmd(nc, in_maps, core_ids=list(range(n_cores)),
                               trace=trace)

    bands = np.empty((4, nsig, n), dtype=np.float32)
    for i in range(n_cores):
        y = np.asarray(res.results[i]["y"]).astype(np.float32)  # [2,128,NHB,N]
        base = i * per_core
        for h in (0, 1):
            r = slice(base + h * 128, base + h * 128 + 128)
            for bi, b in enumerate(HW_BANDS):
                bands[b, r, :] = y[h, :, bi]
    bands[3] = xf - bands[0] - bands[1] - bands[2]
    out = tuple(bands[j].reshape(B, C, n) for j in range(4))
    return out, res


def kernel(x):
    out, _ = run_full(np.asarray(x))
    return out
